# revision 28
# baseline (speedup 1.0000x reference)
"""CompressedFP8Linear on 8 trn2 NeuronCores.

out[B,S,O] = x @ (weight * weight_scale).T + bias
  x:[4,32,8192] f32, weight:[8192,8192] f32 (fp8-e4m3 representable),
  weight_scale:[8192,1] f32, bias:[8192] f16.

Strategy (column-parallel, per sharding hint):
  - Shard weight rows (out_features) across 8 cores; replicate x.
  - The weight values round-trip through fp8-e4m3 in the reference, and
    all |w| <= ~6 << 240 (TRN fp8e4 max normal), so casting the fp32
    weight back to fp8e4 on the host is EXACT and cuts weight DMA 4x:
    8 MiB instead of 32 MiB per core.  x is shipped as bf16 (2 MiB);
    the PE upcasts both operands internally and accumulates fp32.
  - Host-side marshalling (layout only): pack both x and the weight
    shard as [p, kt, free] so every SBUF partition's DMA reads are
    contiguous DRAM runs.
  - Per core: acc = ones^T @ (bias/scale) + sum_k xT.T @ WT, then
    out = acc * scale.  The bias lands in PSUM via a rank-1 seed matmul
    (start=True), so the epilogue is a single vector multiply per
    512-wide half before the output DMA.
  - scale (and bias/scale) rows arrive as [1, O_shard] and are
    broadcast to the 128 token partitions via an exact ones-outer-
    product on the (still idle) PE.
  - No collectives; the host concatenates the 8 output shards.

Memory floor per core: 8 MiB weight (fp8) + 2 MiB x (bf16) + 0.5 MiB
out ~= 30 us at 358 GB/s; PE time ~31 us at bf16 speed — balanced.
"""

import numpy as np
import ml_dtypes

import concourse.bass as bass
import concourse.mybir as mybir
import concourse.tile as tile
from concourse.bass_utils import run_bass_kernel_spmd

B, S, IN, OUT = 4, 32, 8192, 8192
M = B * S                      # 128 tokens
NCORES = 8
OSH = OUT // NCORES            # 1024 out-features per core
KT = IN // 128                 # 64 k-tiles
F32 = mybir.dt.float32
F32R = mybir.dt.float32r
BF16 = mybir.dt.bfloat16
FP8 = mybir.dt.float8e4


def split_waits(nc, max_waits=1):
    """This walrus build encodes at most one sem-wait per instruction;
    move any excess onto NoOps injected just before (same engine queue,
    so ordering semantics are identical)."""
    n = 0
    for f in nc.m.functions:
        for bb in f.blocks:
            out = []
            for inst in bb.instructions:
                si = inst.sync_info
                waits = list(si.on_wait) if si and si.on_wait else []
                if len(waits) > max_waits:
                    extra, keep = waits[:-max_waits], waits[-max_waits:]
                    for i, w in enumerate(extra):
                        out.append(mybir.InstNoOp(
                            name=f"{inst.name}-ws{i}", engine=inst.engine,
                            ins=[], outs=[],
                            sync_info=mybir.SyncInfo(on_wait=[w], on_update=[])))
                        n += 1
                    si.on_wait = keep
                out.append(inst)
            bb.instructions = out
    return n


def build(reps=1, w_engines=("sync", "scalar")):
    """One column-parallel shard: out[128, OSH] = (bias/sc + xT.T@WT) * sc.

    reps > 1 wraps the whole body (including all DMA) in a hardware
    For_i loop for wall-clock timing; the computation is identical each
    rep.  reps == 1 emits the straight-line body (what the harness
    runs).
    """
    nc = bass.Bass()
    # both packed host-side as [p, kt, free]: every partition's slab
    # read is one contiguous DRAM run
    xt_d = nc.dram_tensor("xt", [128, KT, M], BF16, kind="ExternalInput")
    wt_d = nc.dram_tensor("wt", [128, KT, OSH], FP8, kind="ExternalInput")
    sc_d = nc.dram_tensor("scale_r", [1, OSH], F32R, kind="ExternalInput")
    bs_d = nc.dram_tensor("bs_r", [1, OSH], F32R, kind="ExternalInput")
    out_d = nc.dram_tensor("out", [M, OSH], BF16, kind="ExternalOutput")

    # Each dma_start carries ~0.3 us of serial ring overhead, so batch
    # big — but the PE consumes at ~342 GB/s ≈ the HBM cap, so mid-
    # stream slabs stay ~1.5 MiB to avoid bursty arrival deficits.
    # A short first slab gets the PE started early.
    slab_plan = (2, 6, 8, 12, 12, 12, 12)
    assert sum(slab_plan) == KT
    slabs = []
    k0 = 0
    for n in slab_plan:
        slabs.append((k0, n)); k0 += n
    # x chunk plan: 0.25 MiB to unblock the first matmuls, then medium
    # chunks interleaved between weight slabs
    x_plan = (8, 16, 16, 24)
    assert sum(x_plan) == KT

    with tile.TileContext(nc) as tc:
        with (
            tc.tile_pool(name="xp", bufs=2) as xp,
            tc.tile_pool(name="wp", bufs=3) as wp,
            tc.tile_pool(name="cp", bufs=1) as cp,
            tc.tile_pool(name="op", bufs=2) as op,
            tc.tile_pool(name="ps", bufs=2, space="PSUM") as ps,
        ):
            w_engs = [getattr(nc, e) for e in w_engines]

            # broadcast the scale row to all 128 token partitions via an
            # exact ones-outer-product on the PE (f32r single pass); the
            # bias/scale row stays [1, OSH] — it enters as a rank-1
            # matmul seed instead
            ones0 = cp.tile([1, M], F32)
            nc.vector.memset(ones0[:], 1.0)
            ones = cp.tile([1, M], F32R)
            nc.vector.tensor_copy(ones[:], ones0[:])
            ones_r = ones[:]
            scrow = cp.tile([1, OSH], F32R)
            w_engs[1].dma_start(scrow[:], sc_d[:])
            bsrow = cp.tile([1, OSH], F32R)
            w_engs[1].dma_start(bsrow[:], bs_d[:])
            bsrow_r = bsrow[:]
            sc = cp.tile([M, OSH], F32)
            pb = ps.tile([M, OSH], F32, tag="pbcast")
            for og in range(2):
                nc.tensor.matmul(
                    pb[:, og * 512:(og + 1) * 512],
                    ones_r, scrow[:, og * 512:(og + 1) * 512],
                    start=True, stop=True)
            nc.vector.tensor_copy(sc[:], pb[:])

            def body():
                # only sync/scalar may initiate DMAs inside a HW loop
                # (gpsimd's loop branch fails walrus codegen), so x
                # chunks and weight slabs interleave on the two HWDGE
                # rings in need-order
                eng_i = [0]

                def next_eng(nbytes):
                    e = w_engs[eng_i[0] % len(w_engs)]
                    eng_i[0] += 1
                    return e

                xsb = xp.tile([128, KT, M], BF16)
                x_bounds = []
                b = 0
                for n in x_plan:
                    x_bounds.append((b, b + n)); b += n
                x_issued = [0]

                def issue_x_upto(kt_hi):
                    while (x_issued[0] < len(x_bounds)
                           and x_bounds[x_issued[0]][0] < kt_hi):
                        lo, hi = x_bounds[x_issued[0]]
                        next_eng((hi - lo) * 128 * M * 2).dma_start(
                            xsb[:, lo:hi, :], xt_d[:, lo:hi, :])
                        x_issued[0] += 1

                acc0 = ps.tile([M, 512], F32)
                acc1 = ps.tile([M, 512], F32)
                accs = (acc0, acc1)
                # seed PSUM with bias/scale broadcast to every token row
                for og in range(2):
                    nc.tensor.matmul(
                        accs[og][:, :], ones_r,
                        bsrow_r[:, og * 512:(og + 1) * 512],
                        start=True, stop=False, skip_group_check=True)

                outsb = op.tile([M, OSH], BF16)

                def epilogue(og):
                    osl = outsb[:, og * 512:(og + 1) * 512]
                    nc.vector.tensor_mul(osl, accs[og][:, :],
                                         sc[:, og * 512:(og + 1) * 512])
                    # write each half back as soon as its scale is done
                    w_engs[og].dma_start(out_d[:, og * 512:(og + 1) * 512], osl)

                last = len(slabs) - 1
                for t, (k0, n) in enumerate(slabs):
                    issue_x_upto(k0 + n)
                    wsb = wp.tile([128, max(slab_plan), OSH], FP8, tag="wsb")
                    # two half-DMAs per slab: matmuls on the first half
                    # start while the second half streams
                    h = max(1, n // 2)
                    next_eng(h * 128 * OSH).dma_start(
                        wsb[:, :h, :], wt_d[:, k0:k0 + h, :])
                    if n > h:
                        next_eng((n - h) * 128 * OSH).dma_start(
                            wsb[:, h:n, :], wt_d[:, k0 + h:k0 + n, :])
                    if t < last:
                        for s in range(n):
                            k = k0 + s
                            for og in range(2):
                                nc.tensor.matmul(
                                    accs[og][:, :],
                                    xsb[:, k, :],
                                    wsb[:, s, og * 512:(og + 1) * 512],
                                    start=False, stop=False,
                                    skip_group_check=True)
                    else:
                        # last slab og-major: acc0 finishes first, so
                        # its scale+store overlaps acc1's matmuls
                        for og in range(2):
                            for s in range(n):
                                nc.tensor.matmul(
                                    accs[og][:, :],
                                    xsb[:, k0 + s, :],
                                    wsb[:, s, og * 512:(og + 1) * 512],
                                    start=False, stop=(s == n - 1),
                                    skip_group_check=True)
                            epilogue(og)

            if reps == 1:
                body()
            else:
                with tc.For_i(0, reps, staggered_reset=True):
                    body()

    split_waits(nc)
    return nc


def shard_inputs(x, weight, weight_scale, bias):
    """Host-side marshalling into per-core input maps (layout only —
    the fp8 cast is exact because the reference round-trips the weight
    through fp8-e4m3 and |w| << 240)."""
    x = np.asarray(x, dtype=np.float32)
    weight = np.asarray(weight, dtype=np.float32)
    scale = np.asarray(weight_scale, dtype=np.float32).reshape(OUT)
    bias32 = np.asarray(bias).astype(np.float32)
    bs = bias32 / scale

    # pack x as [p, kt, m] bf16 (k = kt*128 + p) so each SBUF partition's
    # x data is one contiguous DRAM run
    xt = np.ascontiguousarray(
        np.transpose(x.reshape(M, KT, 128), (2, 1, 0))).astype(ml_dtypes.bfloat16)
    in_maps = []
    for c in range(NCORES):
        sl = slice(c * OSH, (c + 1) * OSH)
        # weight shard [OSH, IN] -> [p, kt, OSH] fp8, wt[p,t,o] = W[o, t*128+p]
        wt = np.ascontiguousarray(
            np.transpose(weight[sl, :].T.reshape(KT, 128, OSH), (1, 0, 2))
        ).astype(ml_dtypes.float8_e4m3)
        in_maps.append({
            "xt": xt, "wt": wt,
            "scale_r": np.ascontiguousarray(scale[sl][None, :]),
            "bs_r": np.ascontiguousarray(bs[sl][None, :]),
        })
    return in_maps


def kernel(x, weight, weight_scale, bias):
    nc = build(reps=1)
    in_maps = shard_inputs(x, weight, weight_scale, bias)
    res = run_bass_kernel_spmd(nc, in_maps, core_ids=list(range(NCORES)))
    out = np.concatenate(
        [np.asarray(res.results[c]["out"]).astype(np.float32)
         for c in range(NCORES)], axis=1)
    return out.reshape(B, S, OUT)
